# revision 6
# baseline (speedup 1.0000x reference)
"""Trainium2 Bass kernel for the 8-bit SNN barrel shifter.

Reference semantics (all inputs are exactly 0.0/1.0 f32):
    shift = S[:,0] + 2*S[:,1] + 4*S[:,2]
    out[:, i] = P[:, i - shift] if i >= shift else 0

Device strategy (pure data parallel over 8 cores, row-major layout):
  - host repacks P/S to uint8 (0/1 bits) and shards rows across cores
  - per core, the vector engine packs each row's 8 bits into an int16 via a
    shift+add Horner tree, packs the 3 shift bits the same way, applies one
    per-element logical_shift_left, then extracts the 8 output bit planes
  - bit planes are stored planar (uint8); host re-interleaves and casts back
    to f32
"""
import numpy as np

_N = 4194304
_CORES = 8
_NC = _N // _CORES          # rows per core
_PARTS = 128
_R = 1024                   # rows per partition per tile
_T = _NC // (_PARTS * _R)   # tiles per core

_CACHE: dict = {}


def _build(rows_per_core: int, R: int):
    import concourse.tile as tile
    from concourse import bacc, mybir

    dt = mybir.dt
    Alu = mybir.AluOpType
    P = _PARTS
    T = rows_per_core // (P * R)
    assert T * P * R == rows_per_core

    nc = bacc.Bacc("TRN2", target_bir_lowering=False, debug=False)
    p16 = nc.dram_tensor("p16", (rows_per_core, 8), dt.int16, kind="ExternalInput").ap()
    s8 = nc.dram_tensor("s8", (rows_per_core, 4), dt.uint8, kind="ExternalInput").ap()
    o8 = nc.dram_tensor("o8", (T, P, 8, R), dt.uint8, kind="ExternalOutput").ap()

    pr = p16.rearrange("(t p r) c -> t p r c", t=T, p=P, r=R)
    sr = s8.rearrange("(t p r) c -> t p r c", t=T, p=P, r=R)

    with tile.TileContext(nc) as tc:
        with tc.tile_pool(name="io", bufs=3) as io, tc.tile_pool(name="tmp", bufs=2) as tmp:
            for t in range(T):
                pt = io.tile([P, R, 8], dt.int16, tag="p")
                st = io.tile([P, R, 4], dt.uint8, tag="s")
                nc.sync.dma_start(pt[:], pr[t])
                nc.sync.dma_start(st[:], sr[t])

                # pack P bits with contiguous halves (int16 2x mode):
                #   d_i = b_i | b_{i+4}<<4 ; e_j = d_j | d_{j+2}<<2
                #   v = e_0 + 2*e_1
                d = tmp.tile([P, R, 4], dt.int16, tag="d")
                nc.vector.scalar_tensor_tensor(
                    d[:], pt[:, :, 4:8], 4, pt[:, :, 0:4],
                    op0=Alu.logical_shift_left, op1=Alu.bitwise_or)
                e = tmp.tile([P, R, 2], dt.int16, tag="e")
                nc.vector.scalar_tensor_tensor(
                    e[:], d[:, :, 2:4], 2, d[:, :, 0:2],
                    op0=Alu.logical_shift_left, op1=Alu.bitwise_or)
                # arith op casts down to uint8 (v <= 255)
                vi = tmp.tile([P, R], dt.uint8, tag="vi")
                nc.vector.scalar_tensor_tensor(
                    vi[:], e[:, :, 1], 2, e[:, :, 0],
                    op0=Alu.mult, op1=Alu.add)

                # pack S bits: ti = s0 + 2*(s1 + 2*s2)
                a = tmp.tile([P, R], dt.uint8, tag="a")
                nc.vector.scalar_tensor_tensor(
                    a[:], st[:, :, 2], 1, st[:, :, 1],
                    op0=Alu.logical_shift_left, op1=Alu.bitwise_or)
                ti = tmp.tile([P, R], dt.uint8, tag="ti")
                nc.vector.scalar_tensor_tensor(
                    ti[:], a[:], 2, st[:, :, 0],
                    op0=Alu.mult, op1=Alu.add)

                # vs = vi << ti  (per-element shift; uint8 keeps low 8 bits)
                vs = tmp.tile([P, R], dt.uint8, tag="vs")
                nc.vector.tensor_tensor(vs[:], vi[:], ti[:], op=Alu.logical_shift_left)

                # unpack: 8 bit planes, planar layout
                ot = io.tile([P, 8, R], dt.uint8, tag="o")
                for i in range(8):
                    nc.vector.tensor_scalar(
                        ot[:, i, :], vs[:], i, 1,
                        op0=Alu.logical_shift_right, op1=Alu.bitwise_and)

                nc.sync.dma_start(o8[t], ot[:])
    nc.compile()
    _fix_bitwise_imms(nc, mybir)
    return nc


_BITWISE = None


def _fix_bitwise_imms(nc, mybir):
    """walrus requires integer immediates matching the src dtype on bitvec
    tensor_scalar ops; bass emits float32/int32 — rewrite them."""
    global _BITWISE
    Alu = mybir.AluOpType
    if _BITWISE is None:
        _BITWISE = {
            Alu.bitwise_and, Alu.bitwise_or, Alu.bitwise_xor, Alu.bitwise_not,
            Alu.logical_shift_left, Alu.logical_shift_right,
            Alu.arith_shift_left, Alu.arith_shift_right,
        }
    for f in nc.m.functions:
        for blk in f.blocks:
            for i in blk.instructions:
                if type(i).__name__ != "InstTensorScalarPtr":
                    continue
                ops = [getattr(i, "op0", None), getattr(i, "op1", None)]
                if not any(op in _BITWISE for op in ops if op is not None):
                    continue
                src_dt = i.ins[0].dtype
                for k in range(1, len(i.ins)):
                    iv = i.ins[k]
                    if isinstance(iv, mybir.ImmediateValue):
                        i.ins[k] = mybir.ImmediateValue(
                            dtype=src_dt, value=int(iv.value))


def _get_nc():
    key = (_NC, _R)
    if key not in _CACHE:
        _CACHE[key] = _build(*key)
    return _CACHE[key]


def kernel(P: np.ndarray, S: np.ndarray) -> np.ndarray:
    from concourse.bass_utils import run_bass_kernel_spmd

    nc = _get_nc()

    Pb = np.ascontiguousarray(P).astype(np.int16)      # exact 0/1
    s8 = np.zeros((_N, 4), np.uint8)
    s8[:, :3] = np.ascontiguousarray(S).astype(np.uint8)

    in_maps = [
        {"p16": Pb[c * _NC:(c + 1) * _NC], "s8": s8[c * _NC:(c + 1) * _NC]}
        for c in range(_CORES)
    ]
    res = run_bass_kernel_spmd(nc, in_maps, core_ids=list(range(_CORES)))

    out = np.empty((_N, 8), np.float32)
    for c, r in enumerate(res.results):
        o = r["o8"].reshape(_T, _PARTS, 8, _R)
        rows = o.transpose(0, 1, 3, 2).reshape(_NC, 8)
        out[c * _NC:(c + 1) * _NC] = rows
    return out


# revision 9
# speedup vs baseline: 1.2566x; 1.2566x over previous
"""Trainium2 Bass kernel for the 8-bit SNN barrel shifter.

Reference semantics (all inputs are exactly 0.0/1.0 f32):
    shift = S[:,0] + 2*S[:,1] + 4*S[:,2]
    out[:, i] = P[:, i - shift] if i >= shift else 0

Device strategy (pure data parallel over 8 cores, row-major layout):
  - host repacks P/S to uint8 bits (0/1) and shards rows across the 8 cores
  - per core the vector engine packs each row's 8 bit-bytes into one packed
    byte with a bitwise OR-tree over uint32 views (junk bits tracked >= 8),
    packs the 3 shift bits, applies one per-element logical_shift_left, and
    extracts bit pairs with single shift ops (one uint16 lane per 2 output
    bytes; each output byte holds its bit at a known position)
  - host re-interleaves the pair planes, masks the known junk bits, and
    casts back to f32
"""
import numpy as np

_N = 4194304
_CORES = 8
_NC = _N // _CORES          # rows per core
_PARTS = 128
_R = 1024                   # rows per partition per tile
_T = _NC // (_PARTS * _R)   # tiles per core
_POOL_PAIRS = 0             # how many of the 4 pair-extract ops go to GpSimd

_CACHE: dict = {}


def _build(rows_per_core: int, R: int, pool_pairs: int = _POOL_PAIRS):
    import concourse.tile as tile
    from concourse import bacc, mybir

    dt = mybir.dt
    Alu = mybir.AluOpType
    P = _PARTS
    T = rows_per_core // (P * R)
    assert T * P * R == rows_per_core

    nc = bacc.Bacc("TRN2", target_bir_lowering=False, debug=False)
    p8 = nc.dram_tensor("p8", (rows_per_core, 8), dt.uint8, kind="ExternalInput").ap()
    s8 = nc.dram_tensor("s8", (rows_per_core, 4), dt.uint8, kind="ExternalInput").ap()
    o16 = nc.dram_tensor("o16", (T, P, 4, R), dt.uint16, kind="ExternalOutput").ap()

    pr = p8.rearrange("(t p r) c -> t p r c", t=T, p=P, r=R)
    sr = s8.rearrange("(t p r) c -> t p r c", t=T, p=P, r=R)

    with tile.TileContext(nc) as tc:
        with tc.tile_pool(name="io", bufs=3) as io, tc.tile_pool(name="tmp", bufs=2) as tmp:
            for t in range(T):
                pt = io.tile([P, R, 8], dt.uint8, tag="p")
                st = io.tile([P, R, 4], dt.uint8, tag="s")
                nc.sync.dma_start(pt[:], pr[t])
                nc.sync.dma_start(st[:], sr[t])

                # OR-tree over uint16 byte-pair views; real packed bits land
                # at 0..7, junk stays at bits >= 8 and is never extracted
                x16 = pt[:].bitcast(dt.uint16)          # [P, R, 4]
                t1 = tmp.tile([P, R, 4], dt.uint16, tag="t1")
                nc.vector.scalar_tensor_tensor(
                    t1[:], x16, 7, x16,
                    op0=Alu.logical_shift_right, op1=Alu.bitwise_or)
                e = tmp.tile([P, R, 2], dt.uint16, tag="e")
                nc.vector.scalar_tensor_tensor(
                    e[:], t1[:, :, 2:4], 4, t1[:, :, 0:2],
                    op0=Alu.logical_shift_left, op1=Alu.bitwise_or)
                vi = tmp.tile([P, R], dt.uint16, tag="vi")
                nc.vector.scalar_tensor_tensor(
                    vi[:], e[:, :, 1], 2, e[:, :, 0],
                    op0=Alu.logical_shift_left, op1=Alu.bitwise_or)

                # pack S bits: ti = s0 + 2*s1 + 4*s2
                a = tmp.tile([P, R], dt.uint8, tag="a")
                nc.vector.scalar_tensor_tensor(
                    a[:], st[:, :, 2], 1, st[:, :, 1],
                    op0=Alu.logical_shift_left, op1=Alu.bitwise_or)
                ti = tmp.tile([P, R], dt.uint16, tag="ti")
                nc.vector.scalar_tensor_tensor(
                    ti[:], a[:], 2, st[:, :, 0],
                    op0=Alu.mult, op1=Alu.add)

                # vs = vi << ti (per-element shift, uint16)
                vs = tmp.tile([P, R], dt.uint16, tag="vs")
                nc.vector.tensor_tensor(vs[:], vi[:], ti[:], op=Alu.logical_shift_left)

                # extract bit pairs: lane k holds bit 2k at byte0.bit7 and
                # bit 2k+1 at byte1.bit0 (junk elsewhere, host masks)
                ot = io.tile([P, 4, R], dt.uint16, tag="o")
                for k in range(4):
                    eng = nc.gpsimd if k < pool_pairs else nc.vector
                    eng.tensor_scalar(
                        ot[:, k, :], vs[:], 7 - 2 * k, None,
                        op0=Alu.logical_shift_left)

                nc.sync.dma_start(o16[t], ot[:])
    nc.compile()
    _fix_bitwise_imms(nc, mybir)
    return nc


_BITWISE = None


def _fix_bitwise_imms(nc, mybir):
    """walrus requires integer immediates matching the src dtype on bitvec
    tensor_scalar ops; bass emits float32/int32 — rewrite them."""
    global _BITWISE
    Alu = mybir.AluOpType
    if _BITWISE is None:
        _BITWISE = {
            Alu.bitwise_and, Alu.bitwise_or, Alu.bitwise_xor, Alu.bitwise_not,
            Alu.logical_shift_left, Alu.logical_shift_right,
            Alu.arith_shift_left, Alu.arith_shift_right,
        }
    for f in nc.m.functions:
        for blk in f.blocks:
            for i in blk.instructions:
                if type(i).__name__ != "InstTensorScalarPtr":
                    continue
                ops = [getattr(i, "op0", None), getattr(i, "op1", None)]
                if not any(op in _BITWISE for op in ops if op is not None):
                    continue
                src_dt = i.ins[0].dtype
                for k in range(1, len(i.ins)):
                    iv = i.ins[k]
                    if isinstance(iv, mybir.ImmediateValue):
                        i.ins[k] = mybir.ImmediateValue(
                            dtype=src_dt, value=int(iv.value))


def _get_nc():
    key = (_NC, _R, _S_MODE)
    if key not in _CACHE:
        _CACHE[key] = _build(*key)
    return _CACHE[key]


def _prep_inputs(P, S):
    Pb = np.ascontiguousarray(P).astype(np.uint8)      # exact 0/1
    s8 = np.zeros((P.shape[0], 4), np.uint8)
    s8[:, :3] = np.ascontiguousarray(S).astype(np.uint8)
    return Pb, s8


def _unshard_out(o16_list):
    out = np.empty((_N, 8), np.float32)
    for c, r in enumerate(o16_list):
        o = r.reshape(_T, _PARTS, 4, _R)
        b = o.view(np.uint8).reshape(_T, _PARTS, 4, _R, 2)
        rows = np.empty((_T, _PARTS, _R, 8), np.uint8)
        rows[..., 0::2] = ((b[..., 0] >> 7) & 1).transpose(0, 1, 3, 2)
        rows[..., 1::2] = (b[..., 1] & 1).transpose(0, 1, 3, 2)
        out[c * _NC:(c + 1) * _NC] = rows.reshape(_NC, 8)
    return out


def kernel(P: np.ndarray, S: np.ndarray) -> np.ndarray:
    from concourse.bass_utils import run_bass_kernel_spmd

    nc = _get_nc()
    Pb, s8 = _prep_inputs(P, S)
    in_maps = [
        {"p8": Pb[c * _NC:(c + 1) * _NC], "s8": s8[c * _NC:(c + 1) * _NC]}
        for c in range(_CORES)
    ]
    res = run_bass_kernel_spmd(nc, in_maps, core_ids=list(range(_CORES)))
    return _unshard_out([r["o16"] for r in res.results])


# revision 10
# speedup vs baseline: 1.5763x; 1.2544x over previous
"""Trainium2 Bass kernel for the 8-bit SNN barrel shifter.

Reference semantics (all inputs are exactly 0.0/1.0 f32):
    shift = S[:,0] + 2*S[:,1] + 4*S[:,2]
    out[:, i] = P[:, i - shift] if i >= shift else 0

Device strategy (pure data parallel over 8 cores, row-major layout):
  - host repacks P/S to uint8 bits (0/1) and shards rows across the 8 cores
  - per core the vector engine packs each row's 8 bit-bytes into one packed
    byte with a bitwise OR-tree over uint32 views (junk bits tracked >= 8),
    packs the 3 shift bits, applies one per-element logical_shift_left, and
    extracts bit pairs with single shift ops (one uint16 lane per 2 output
    bytes; each output byte holds its bit at a known position)
  - host re-interleaves the pair planes, masks the known junk bits, and
    casts back to f32
"""
import numpy as np

_N = 4194304
_CORES = 8
_NC = _N // _CORES          # rows per core
_PARTS = 128
_R = 1024                   # rows per partition per tile
_T = _NC // (_PARTS * _R)   # tiles per core
_POOL_PAIRS = 0             # how many of the 4 pair-extract ops go to GpSimd

_CACHE: dict = {}


def _build(rows_per_core: int, R: int, pool_pairs: int = _POOL_PAIRS):
    import concourse.tile as tile
    from concourse import bacc, mybir

    dt = mybir.dt
    Alu = mybir.AluOpType
    P = _PARTS
    T = rows_per_core // (P * R)
    assert T * P * R == rows_per_core

    nc = bacc.Bacc("TRN2", target_bir_lowering=False, debug=False)
    p8 = nc.dram_tensor("p8", (rows_per_core, 8), dt.uint8, kind="ExternalInput").ap()
    s8 = nc.dram_tensor("s8", (rows_per_core, 4), dt.uint8, kind="ExternalInput").ap()
    o16 = nc.dram_tensor("o16", (T, P, 4, R), dt.uint16, kind="ExternalOutput").ap()

    pr = p8.rearrange("(t p r) c -> t p r c", t=T, p=P, r=R)
    sr = s8.rearrange("(t p r) c -> t p r c", t=T, p=P, r=R)

    with tile.TileContext(nc) as tc:
        with tc.tile_pool(name="io", bufs=3) as io, tc.tile_pool(name="tmp", bufs=2) as tmp:
            for t in range(T):
                pt = io.tile([P, R, 8], dt.uint8, tag="p")
                st = io.tile([P, R, 4], dt.uint8, tag="s")
                nc.sync.dma_start(pt[:], pr[t])
                nc.sync.dma_start(st[:], sr[t])

                # pack via interleave-then-fold over uint16 byte-pair views:
                #   x16_i = b_2i | b_{2i+1}<<8
                #   u_j = x16_j | x16_{j+2}<<4   -> evens of u at bits {0,4},
                #                                   odds at {8,12}
                #   w   = u_0 | u_1<<2           -> evens at {0,2,4,6},
                #                                   odds at {8,10,12,14}
                #   vi  = w | w>>7               -> bits 0..7 packed; junk >= 8
                x16 = pt[:].bitcast(dt.uint16)          # [P, R, 4]
                u = tmp.tile([P, R, 2], dt.uint16, tag="u")
                nc.vector.scalar_tensor_tensor(
                    u[:], x16[:, :, 2:4], 4, x16[:, :, 0:2],
                    op0=Alu.logical_shift_left, op1=Alu.bitwise_or)
                w = tmp.tile([P, R], dt.uint16, tag="w")
                nc.vector.scalar_tensor_tensor(
                    w[:], u[:, :, 1], 2, u[:, :, 0],
                    op0=Alu.logical_shift_left, op1=Alu.bitwise_or)
                vi = tmp.tile([P, R], dt.uint16, tag="vi")
                nc.vector.scalar_tensor_tensor(
                    vi[:], w[:], 7, w[:],
                    op0=Alu.logical_shift_right, op1=Alu.bitwise_or)

                # pack S bits: ti = s0 + 2*s1 + 4*s2
                a = tmp.tile([P, R], dt.uint8, tag="a")
                nc.vector.scalar_tensor_tensor(
                    a[:], st[:, :, 2], 1, st[:, :, 1],
                    op0=Alu.logical_shift_left, op1=Alu.bitwise_or)
                ti = tmp.tile([P, R], dt.uint16, tag="ti")
                nc.vector.scalar_tensor_tensor(
                    ti[:], a[:], 2, st[:, :, 0],
                    op0=Alu.mult, op1=Alu.add)

                # vs = vi << ti (per-element shift, uint16)
                vs = tmp.tile([P, R], dt.uint16, tag="vs")
                nc.vector.tensor_tensor(vs[:], vi[:], ti[:], op=Alu.logical_shift_left)

                # extract bit pairs: lane k holds bit 2k at byte0.bit7 and
                # bit 2k+1 at byte1.bit0 (junk elsewhere, host masks)
                ot = io.tile([P, 4, R], dt.uint16, tag="o")
                for k in range(4):
                    eng = nc.gpsimd if k < pool_pairs else nc.vector
                    eng.tensor_scalar(
                        ot[:, k, :], vs[:], 7 - 2 * k, None,
                        op0=Alu.logical_shift_left)

                nc.sync.dma_start(o16[t], ot[:])
    nc.compile()
    _fix_bitwise_imms(nc, mybir)
    return nc


_BITWISE = None


def _fix_bitwise_imms(nc, mybir):
    """walrus requires integer immediates matching the src dtype on bitvec
    tensor_scalar ops; bass emits float32/int32 — rewrite them."""
    global _BITWISE
    Alu = mybir.AluOpType
    if _BITWISE is None:
        _BITWISE = {
            Alu.bitwise_and, Alu.bitwise_or, Alu.bitwise_xor, Alu.bitwise_not,
            Alu.logical_shift_left, Alu.logical_shift_right,
            Alu.arith_shift_left, Alu.arith_shift_right,
        }
    for f in nc.m.functions:
        for blk in f.blocks:
            for i in blk.instructions:
                if type(i).__name__ != "InstTensorScalarPtr":
                    continue
                ops = [getattr(i, "op0", None), getattr(i, "op1", None)]
                if not any(op in _BITWISE for op in ops if op is not None):
                    continue
                src_dt = i.ins[0].dtype
                for k in range(1, len(i.ins)):
                    iv = i.ins[k]
                    if isinstance(iv, mybir.ImmediateValue):
                        i.ins[k] = mybir.ImmediateValue(
                            dtype=src_dt, value=int(iv.value))


def _get_nc():
    key = (_NC, _R, _S_MODE)
    if key not in _CACHE:
        _CACHE[key] = _build(*key)
    return _CACHE[key]


def _prep_inputs(P, S):
    Pb = np.ascontiguousarray(P).astype(np.uint8)      # exact 0/1
    s8 = np.zeros((P.shape[0], 4), np.uint8)
    s8[:, :3] = np.ascontiguousarray(S).astype(np.uint8)
    return Pb, s8


def _unshard_out(o16_list):
    out = np.empty((_N, 8), np.float32)
    for c, r in enumerate(o16_list):
        o = r.reshape(_T, _PARTS, 4, _R)
        b = o.view(np.uint8).reshape(_T, _PARTS, 4, _R, 2)
        rows = np.empty((_T, _PARTS, _R, 8), np.uint8)
        rows[..., 0::2] = ((b[..., 0] >> 7) & 1).transpose(0, 1, 3, 2)
        rows[..., 1::2] = (b[..., 1] & 1).transpose(0, 1, 3, 2)
        out[c * _NC:(c + 1) * _NC] = rows.reshape(_NC, 8)
    return out


def kernel(P: np.ndarray, S: np.ndarray) -> np.ndarray:
    from concourse.bass_utils import run_bass_kernel_spmd

    nc = _get_nc()
    Pb, s8 = _prep_inputs(P, S)
    in_maps = [
        {"p8": Pb[c * _NC:(c + 1) * _NC], "s8": s8[c * _NC:(c + 1) * _NC]}
        for c in range(_CORES)
    ]
    res = run_bass_kernel_spmd(nc, in_maps, core_ids=list(range(_CORES)))
    return _unshard_out([r["o16"] for r in res.results])


# revision 14
# speedup vs baseline: 1.5820x; 1.0036x over previous
"""Trainium2 Bass kernel for the 8-bit SNN barrel shifter.

Reference semantics (all inputs are exactly 0.0/1.0 f32):
    shift = S[:,0] + 2*S[:,1] + 4*S[:,2]
    out[:, i] = P[:, i - shift] if i >= shift else 0

Device strategy (pure data parallel over 8 cores, row-major layout):
  - host repacks P/S to uint8 bits (0/1) and shards rows across the 8 cores
  - per core the vector engine packs each row's 8 bit-bytes into one packed
    byte with a bitwise OR-tree over uint32 views (junk bits tracked >= 8),
    packs the 3 shift bits, applies one per-element logical_shift_left, and
    extracts bit pairs with single shift ops (one uint16 lane per 2 output
    bytes; each output byte holds its bit at a known position)
  - host re-interleaves the pair planes, masks the known junk bits, and
    casts back to f32
"""
import numpy as np

_N = 4194304
_CORES = 8
_NC = _N // _CORES          # rows per core
_PARTS = 128
_R = 1024                   # rows per partition per tile
_T = _NC // (_PARTS * _R)   # tiles per core
_POOL_PAIRS = 0             # how many of the 4 pair-extract ops go to GpSimd

_CACHE: dict = {}


def _build(rows_per_core: int, R, pool_pairs: int = _POOL_PAIRS, bufs: int = 3):
    import concourse.tile as tile
    from concourse import bacc, mybir

    dt = mybir.dt
    Alu = mybir.AluOpType
    P = _PARTS
    rpp = rows_per_core // P          # rows per partition
    rs = [R] * (rpp // R) if isinstance(R, int) else list(R)
    assert sum(rs) == rpp

    nc = bacc.Bacc("TRN2", target_bir_lowering=False, debug=False)
    p8 = nc.dram_tensor("p8", (rows_per_core, 8), dt.uint8, kind="ExternalInput").ap()
    s8 = nc.dram_tensor("s8", (rows_per_core, 4), dt.uint8, kind="ExternalInput").ap()
    o16 = nc.dram_tensor("o16", (rows_per_core * 4,), dt.uint16,
                         kind="ExternalOutput").ap()

    pr = p8.rearrange("(p r) c -> p r c", p=P, r=rpp)
    sr = s8.rearrange("(p r) c -> p r c", p=P, r=rpp)

    with tile.TileContext(nc) as tc:
        with tc.tile_pool(name="io", bufs=bufs) as io, tc.tile_pool(name="tmp", bufs=2) as tmp:
            r0 = 0
            for R in rs:
                pt = io.tile([P, R, 8], dt.uint8, tag="p")
                st = io.tile([P, R, 4], dt.uint8, tag="s")
                nc.sync.dma_start(pt[:], pr[:, r0:r0 + R])
                nc.sync.dma_start(st[:], sr[:, r0:r0 + R])

                # pack via interleave-then-fold over uint16 byte-pair views:
                #   x16_i = b_2i | b_{2i+1}<<8
                #   u_j = x16_j | x16_{j+2}<<4   -> evens of u at bits {0,4},
                #                                   odds at {8,12}
                #   w   = u_0 | u_1<<2           -> evens at {0,2,4,6},
                #                                   odds at {8,10,12,14}
                #   vi  = w | w>>7               -> bits 0..7 packed; junk >= 8
                x16 = pt[:].bitcast(dt.uint16)          # [P, R, 4]
                u = tmp.tile([P, R, 2], dt.uint16, tag="u")
                nc.vector.scalar_tensor_tensor(
                    u[:], x16[:, :, 2:4], 4, x16[:, :, 0:2],
                    op0=Alu.logical_shift_left, op1=Alu.bitwise_or)
                w = tmp.tile([P, R], dt.uint16, tag="w")
                nc.vector.scalar_tensor_tensor(
                    w[:], u[:, :, 1], 2, u[:, :, 0],
                    op0=Alu.logical_shift_left, op1=Alu.bitwise_or)
                vi = tmp.tile([P, R], dt.uint16, tag="vi")
                nc.vector.scalar_tensor_tensor(
                    vi[:], w[:], 7, w[:],
                    op0=Alu.logical_shift_right, op1=Alu.bitwise_or)

                # pack S bits: ti = s0 + 2*s1 + 4*s2
                a = tmp.tile([P, R], dt.uint8, tag="a")
                nc.vector.scalar_tensor_tensor(
                    a[:], st[:, :, 2], 1, st[:, :, 1],
                    op0=Alu.logical_shift_left, op1=Alu.bitwise_or)
                ti = tmp.tile([P, R], dt.uint16, tag="ti")
                nc.vector.scalar_tensor_tensor(
                    ti[:], a[:], 2, st[:, :, 0],
                    op0=Alu.mult, op1=Alu.add)

                # vs = vi << ti (per-element shift, uint16)
                vs = tmp.tile([P, R], dt.uint16, tag="vs")
                nc.vector.tensor_tensor(vs[:], vi[:], ti[:], op=Alu.logical_shift_left)

                # extract bit pairs: lane k holds bit 2k at byte0.bit7 and
                # bit 2k+1 at byte1.bit0 (junk elsewhere, host masks)
                ot = io.tile([P, 4, R], dt.uint16, tag="o")
                for k in range(4):
                    eng = nc.gpsimd if k < pool_pairs else nc.vector
                    eng.tensor_scalar(
                        ot[:, k, :], vs[:], 7 - 2 * k, None,
                        op0=Alu.logical_shift_left)

                dst = o16[4 * P * r0: 4 * P * (r0 + R)].rearrange(
                    "(p c r) -> p c r", p=P, c=4, r=R)
                nc.sync.dma_start(dst, ot[:])
                r0 += R
    nc.compile()
    _fix_bitwise_imms(nc, mybir)
    return nc


_BITWISE = None


def _fix_bitwise_imms(nc, mybir):
    """walrus requires integer immediates matching the src dtype on bitvec
    tensor_scalar ops; bass emits float32/int32 — rewrite them."""
    global _BITWISE
    Alu = mybir.AluOpType
    if _BITWISE is None:
        _BITWISE = {
            Alu.bitwise_and, Alu.bitwise_or, Alu.bitwise_xor, Alu.bitwise_not,
            Alu.logical_shift_left, Alu.logical_shift_right,
            Alu.arith_shift_left, Alu.arith_shift_right,
        }
    for f in nc.m.functions:
        for blk in f.blocks:
            for i in blk.instructions:
                if type(i).__name__ != "InstTensorScalarPtr":
                    continue
                ops = [getattr(i, "op0", None), getattr(i, "op1", None)]
                if not any(op in _BITWISE for op in ops if op is not None):
                    continue
                src_dt = i.ins[0].dtype
                for k in range(1, len(i.ins)):
                    iv = i.ins[k]
                    if isinstance(iv, mybir.ImmediateValue):
                        i.ins[k] = mybir.ImmediateValue(
                            dtype=src_dt, value=int(iv.value))


def _get_nc():
    key = (_NC, tuple(_R) if not isinstance(_R, int) else _R)
    if key not in _CACHE:
        _CACHE[key] = _build(*key)
    return _CACHE[key]


def _prep_inputs(P, S):
    Pb = np.ascontiguousarray(P).astype(np.uint8)      # exact 0/1
    s8 = np.zeros((P.shape[0], 4), np.uint8)
    s8[:, :3] = np.ascontiguousarray(S).astype(np.uint8)
    return Pb, s8


def _unshard_core(o16, rows_per_core, R):
    P = _PARTS
    rpp = rows_per_core // P
    rs = [R] * (rpp // R) if isinstance(R, int) else list(R)
    rows = np.empty((P, rpp, 8), np.uint8)
    r0 = 0
    for Rt in rs:
        chunk = o16[4 * P * r0: 4 * P * (r0 + Rt)].reshape(P, 4, Rt)
        b = chunk.view(np.uint8).reshape(P, 4, Rt, 2)
        rows[:, r0:r0 + Rt, 0::2] = ((b[..., 0] >> 7) & 1).transpose(0, 2, 1)
        rows[:, r0:r0 + Rt, 1::2] = (b[..., 1] & 1).transpose(0, 2, 1)
        r0 += Rt
    return rows.reshape(rows_per_core, 8)


def _unshard_out(o16_list):
    out = np.empty((_N, 8), np.float32)
    for c, r in enumerate(o16_list):
        out[c * _NC:(c + 1) * _NC] = _unshard_core(r.ravel(), _NC, _R)
    return out


def kernel(P: np.ndarray, S: np.ndarray) -> np.ndarray:
    from concourse.bass_utils import run_bass_kernel_spmd

    nc = _get_nc()
    Pb, s8 = _prep_inputs(P, S)
    in_maps = [
        {"p8": Pb[c * _NC:(c + 1) * _NC], "s8": s8[c * _NC:(c + 1) * _NC]}
        for c in range(_CORES)
    ]
    res = run_bass_kernel_spmd(nc, in_maps, core_ids=list(range(_CORES)))
    return _unshard_out([r["o16"] for r in res.results])
